# revision 4
# baseline (speedup 1.0000x reference)
"""Corner-detection (structure-tensor min-eigenvalue + edge magnitude)
Bass/Tile kernel for Trainium2, v5.

Layout: the 4096-row image is covered by 33 vertical tiles of 128 input
rows: tile 0 yields valid output rows [0,126), interior tile j yields
[124j+2, 124j+126), tile 32 (input rows [3968,4096)) yields [3970,4096).
124*31 + 126*2 = 4096 exactly.  Core k owns tiles 4k..4k+3 (two
2048-col stripes each) plus a 512-col slice of tile 32 (the "rail"),
so every core does 4.125 tile-units of work (vs 5 for 512-row bands).

Per (tile, stripe) pipeline:
  DMA : x01,x2 [128,2056] f16 (host-padded; host packs channels 0,1 into
        x01 = (w0*x0 + w1*x1)/w2 during the f16 conversion pass)
  DVE : gray' = x01 + x2  (= gray/w2; w2 folded into the grad weights)
  PE  : ix = 0.5*Ix, iy = 0.5*Iy directly from gray' via 2+3 shifted-rhs
        matmuls with banded lhsT (vertical taps in the weights, horizontal
        taps as rhs column shifts; 0.5*w2 folded into the weights),
        chunked 512 into a 2-bank PSUM tile (ix | iy).
  ACT : one copy per chunk-pair evacuates ix|iy PSUM -> SBUF f16.
  DVE : pxx=ix^2 pyy=iy^2 pxy=ix*iy q1=pxx+pyy q2=pxx-pyy (tt, 2x mode);
        aix=|ix| aiy=|iy| (tensor_scalar abs_max, 4x mode).
  GP  : edge = aix + aiy.
  PE  : TR=4*box(q1), DF=4*box(q2), C2=8*box(pxy) via banded lhsT x 3
        shifted rhs; DF,C2 accumulate into one 2-bank PSUM tile.
  ACT : ddcq = (DF^2 | C2^2) in one Square from PSUM; ss = sqrt(ee).
  DVE : ee = dd + cq; eig = TR - ss (TR read from PSUM).
Rail: same pipeline at 512+halo cols; image-boundary columns are zeroed
via a per-core mask on ix/iy (the only core-dependent data), image
boundary rows/cols elsewhere fall out of the banded weights and two
fixed column trims on the box taps (zero-pad conv semantics).
"""

import numpy as np

# ---------------------------------------------------------------------------
# BIR patch: this container's walrus build accepts only ONE sync-wait per
# instruction, but Tile's kernel-tail Drain aggregates one wait per logical
# processor.  Split any instruction carrying >1 waits into preceding
# same-engine Drain clones each carrying a single wait.
# ---------------------------------------------------------------------------
import orjson

_MAX_WAITS = 1


def _split_block(insts):
    out = []
    ctr = 0
    for inst in insts:
        si = inst.get("sync_info")
        ow = (si or {}).get("on_wait") or []
        if len(ow) > _MAX_WAITS:
            extra, keep = ow[:-_MAX_WAITS], ow[-_MAX_WAITS:]
            for i in range(0, len(extra), _MAX_WAITS):
                out.append(
                    {
                        "name": f"{inst['name']}-ws{ctr}",
                        "opcode": "Drain",
                        "engine": inst["engine"],
                        "ins": [],
                        "outs": [],
                        "is_reset_sema": False,
                        "debug": inst.get("debug", 0),
                        "sync_info": {
                            "on_update": [],
                            "on_wait": extra[i : i + _MAX_WAITS],
                        },
                    }
                )
                ctr += 1
            si["on_wait"] = keep
        out.append(inst)
    return out


def _split_sem_waits(bir_json: bytes) -> bytes:
    d = orjson.loads(bir_json)
    changed = False
    for fn in d.get("functions", []):
        for blk in fn.get("blocks", []):
            insts = blk.get("instructions", [])
            if any(
                len(((i.get("sync_info") or {}).get("on_wait") or [])) > _MAX_WAITS
                for i in insts
            ):
                blk["instructions"] = _split_block(insts)
                changed = True
    return orjson.dumps(d) if changed else bir_json


def _install_birpatch():
    import concourse.bass_utils as bu
    import concourse.bass2jax as b2j

    if getattr(bu.compile_bir_kernel, "_waitsplit", False):
        return

    orig = bu.compile_bir_kernel

    def patched(bir_json, tmpdir, neff_name="file.neff"):
        return orig(_split_sem_waits(bir_json), tmpdir, neff_name)

    patched._waitsplit = True
    bu.compile_bir_kernel = patched
    b2j.compile_bir_kernel = patched


_install_birpatch()

import concourse.bass as bass
import concourse.tile as tile
from concourse import mybir
from concourse.bass_utils import run_bass_kernel_spmd

# ---------------------------------------------------------------------------
# Geometry
# ---------------------------------------------------------------------------
N_CORES = 8
H = W = 4096
NT = 4                  # full tiles per core
PW = W + 8              # host-padded width; image col c at pad col c+2
SW = 2056               # stripe buffer width
SIMG = 2048             # image cols per stripe
NSTRIPE = 2
RW = 520                # rail buffer width (image cols 512k-2 .. 512k+518)
GRAD_CHUNKS = [(0, 512), (512, 1024), (1024, 1536), (1536, 2048), (2048, 2050)]
BOX_CHUNKS = [(0, 512), (512, 1024), (1024, 1536), (1536, 2048)]
R_GRAD_CHUNKS = [(0, 512), (512, 514)]
R_BOX_CHUNKS = [(0, 512)]

GRAY = np.array([0.2989, 0.587, 0.114], dtype=np.float64)
S0 = float(GRAY[0] / GRAY[1])        # gray' = (s0*x0 + x1)*...
S1 = float(GRAY[1] / GRAY[2])        # gray' = s1*g1 + x2 = gray/w2
ALPHA = 0.5 * float(GRAY[2])         # folded into grad weights
SMOOTH = np.array([3.0, 10.0, 3.0], dtype=np.float64) / 16.0
INTERP = np.array([1.0, 0.0, -1.0], dtype=np.float64)

F32 = mybir.dt.float32
F16 = mybir.dt.float16
ALU = mybir.AluOpType

WEIGHT_NAMES = ["wixp", "wixm", "wiya", "wiyb", "box4", "box8"]


def _band(coeffs, scale):
    """lhsT[k=m+dk, m] = coeffs[dk+1]*scale, |dk|<=1.
    out[m] = sum_a coeffs[a] * rhs[m+a-1]  (3-tap vertical correlation,
    implicit zero-pad at partition edges)."""
    w = np.zeros((128, 128), dtype=np.float64)
    for m in range(128):
        for dk in (-1, 0, 1):
            k = m + dk
            if 0 <= k < 128:
                w[k, m] = coeffs[dk + 1] * scale
    return w.astype(np.float16)


def _weights():
    ones = np.array([1.0, 1.0, 1.0])
    return {
        "wixp": _band(SMOOTH, ALPHA),
        "wixm": _band(SMOOTH, -ALPHA),
        "wiya": _band(INTERP, ALPHA * 3.0 / 16.0),
        "wiyb": _band(INTERP, ALPHA * 10.0 / 16.0),
        "box4": _band(ones, 4.0),
        "box8": _band(ones, 8.0),
    }


# ---------------------------------------------------------------------------
# Kernel build
# ---------------------------------------------------------------------------
def build_nc(repeats=1, mode="full", gray_eng="dve", edge_eng="gp",
             xbufs=2, gbufs=2):
    from contextlib import ExitStack
    import os

    nc = bass.Bass("TRN2", target_bir_lowering=False, num_devices=N_CORES)
    xs = nc.declare_dram_parameter("xs", [2, NT, 128, PW], F16, isOutput=False)
    xr = nc.declare_dram_parameter("xr", [2, 128, RW], F16, isOutput=False)
    rmask = nc.declare_dram_parameter("rmask", [128, RW], F16, isOutput=False)
    wt = {}
    for name in WEIGHT_NAMES:
        wt[name] = nc.declare_dram_parameter(name, [128, 128], F16, isOutput=False)
    edge_o = nc.declare_dram_parameter(
        "edge", [NT, NSTRIPE, 128, SIMG], F16, isOutput=True)
    eig_o = nc.declare_dram_parameter(
        "eig", [NT, NSTRIPE, 128, SIMG], F16, isOutput=True)
    redge_o = nc.declare_dram_parameter("redge", [128, 512], F16, isOutput=True)
    reig_o = nc.declare_dram_parameter("reig", [128, 512], F16, isOutput=True)

    with ExitStack() as ctx:
        tc = ctx.enter_context(
            tile.TileContext(nc, trace_sim=bool(os.environ.get("KERNEL_TRACE_SIM")))
        )
        singles = ctx.enter_context(tc.tile_pool(name="singles", bufs=1))
        xpool = ctx.enter_context(tc.tile_pool(name="x", bufs=xbufs))
        gpool = ctx.enter_context(tc.tile_pool(name="g", bufs=gbufs))
        prod = ctx.enter_context(tc.tile_pool(name="prod", bufs=2))
        outp = ctx.enter_context(tc.tile_pool(name="outp", bufs=2))
        tailp = ctx.enter_context(tc.tile_pool(name="tail", bufs=3))
        psg = ctx.enter_context(tc.tile_pool(name="psg", bufs=2, space="PSUM"))
        psb = ctx.enter_context(tc.tile_pool(name="psb", bufs=1, space="PSUM"))

        wsb = {}
        for name in WEIGHT_NAMES:
            t = singles.tile([128, 128], F16, name=name, tag=name)
            nc.sync.dma_start(out=t[:], in_=wt[name][:, :])
            wsb[name] = t
        rmask_sb = singles.tile([128, RW], F16, name="rmask_sb", tag="rmask_sb")
        nc.sync.dma_start(out=rmask_sb[:], in_=rmask[:, :])
        if mode == "dmaonly":
            zed = singles.tile([128, SW], F16, name="zed", tag="zed")
            nc.vector.memset(zed[:], 0.0)

        def _tt(eng, out, a, b, op):
            (nc.gpsimd if eng == "gp" else nc.vector).tensor_tensor(
                out, a, b, op)

        def grads(gray_t, bw, chunks, gxy_t):
            """PE: ix|iy from gray' into 2-bank PSUM; ACT evac to gxy."""
            for lo, hi in chunks:
                n = hi - lo
                gps = psg.tile([128, 2, 512], F32, tag="grad_ps", name="grad_ps")
                nc.tensor.matmul(gps[:, 0, :n], wsb["wixp"][:],
                                 gray_t[:, lo:hi], start=True, stop=False)
                nc.tensor.matmul(gps[:, 0, :n], wsb["wixm"][:],
                                 gray_t[:, lo + 2:hi + 2], start=False, stop=True)
                nc.tensor.matmul(gps[:, 1, :n], wsb["wiya"][:],
                                 gray_t[:, lo:hi], start=True, stop=False)
                nc.tensor.matmul(gps[:, 1, :n], wsb["wiya"][:],
                                 gray_t[:, lo + 2:hi + 2], start=False, stop=False)
                nc.tensor.matmul(gps[:, 1, :n], wsb["wiyb"][:],
                                 gray_t[:, lo + 1:hi + 1], start=False, stop=True)
                nc.scalar.copy(out=gxy_t[:, :, lo:hi], in_=gps[:, :, :n])

        def box_tail(q1_t, q2_t, pxy_t, chunks, eig_t, s2q1_t=None):
            """PE box chains + ACT/DVE tail into eig_t[lo:hi].
            Out-of-image product columns are pre-zeroed (memset/mask), so
            all taps run full range.  If s2q1_t is given (= q1[m]+q1[m+2]
            pre-summed on Pool), the TR chain needs only 2 matmuls."""
            for lo, hi in chunks:
                n = hi - lo
                tr = psb.tile([128, 512], F32, tag="tr_ps", name="tr_ps", bufs=2)
                dfc = psb.tile([128, 2, 512], F32, tag="dfc_ps", name="dfc_ps")
                if s2q1_t is not None:
                    nc.tensor.matmul(tr[:, :n], wsb["box4"][:],
                                     s2q1_t[:, lo:hi],
                                     start=True, stop=False,
                                     skip_group_check=True)
                    nc.tensor.matmul(tr[:, :n], wsb["box4"][:],
                                     q1_t[:, lo + 1:hi + 1],
                                     start=False, stop=True,
                                     skip_group_check=True)
                    chains = ((dfc[:, 0, :n], q2_t, "box4"),
                              (dfc[:, 1, :n], pxy_t, "box8"))
                else:
                    chains = ((tr[:, :n], q1_t, "box4"),
                              (dfc[:, 0, :n], q2_t, "box4"),
                              (dfc[:, 1, :n], pxy_t, "box8"))
                for ps, q, w in chains:
                    for i, d in enumerate((0, -1, 1)):
                        nc.tensor.matmul(
                            ps[:, :n], wsb[w][:],
                            q[:, lo + d + 1:hi + d + 1],
                            start=(i == 0), stop=(i == 2),
                            skip_group_check=True,
                        )
                ddcq = tailp.tile([128, 2, 512], F16, tag="ddcq", name="ddcq")
                ee = tailp.tile([128, 512], F16, tag="ee", name="ee")
                ss = tailp.tile([128, 512], F16, tag="ss", name="ss")
                nc.scalar.square(out=ddcq[:, :, :n], in_=dfc[:, :, :n])
                nc.vector.tensor_tensor(
                    ee[:, :n], ddcq[:, 0, :n], ddcq[:, 1, :n], ALU.add)
                nc.scalar.sqrt(out=ss[:, :n], in_=ee[:, :n])
                nc.vector.tensor_tensor(
                    eig_t[:, lo:hi], tr[:, :n], ss[:, :n], ALU.subtract)

        for _rep in range(repeats):
            # ----------------- rail: 512-col slice of tile 32 -------------
            xrt = [xpool.tile([128, RW], F16, tag=f"xr{c}", name=f"xr{c}")
                   for c in range(2)]
            for c in range(2):
                nc.sync.dma_start(out=xrt[c][:], in_=xr[c, :, :])
            if mode == "dmaonly":
                nc.sync.dma_start(out=redge_o[:, :], in_=zed[:, 0:512])
                nc.sync.dma_start(out=reig_o[:, :], in_=zed[:, 0:512])
            else:
                # host pre-scales x0,x1 by w0/w2, w1/w2: gray' = x0'+x1'+x2
                grayr = gpool.tile([128, RW], F16, tag="grayr", name="grayr")
                _tt(gray_eng, grayr[:, 0:518], xrt[0][:, 0:518],
                    xrt[1][:, 0:518], ALU.add)
                gxyr = gpool.tile([128, 2, RW], F16, tag="gxyr", name="gxyr")
                grads(grayr, wsb, R_GRAD_CHUNKS, gxyr)
                ixm = prod.tile([128, RW], F16, tag="ixm", name="ixm")
                iym = prod.tile([128, RW], F16, tag="iym", name="iym")
                nc.vector.tensor_tensor(
                    ixm[:, 0:514], gxyr[:, 0, 0:514], rmask_sb[:, 0:514],
                    ALU.mult)
                nc.vector.tensor_tensor(
                    iym[:, 0:514], gxyr[:, 1, 0:514], rmask_sb[:, 0:514],
                    ALU.mult)
                pxxr = prod.tile([128, RW], F16, tag="pxxr", name="pxxr")
                pyyr = prod.tile([128, RW], F16, tag="pyyr", name="pyyr")
                pxyr = prod.tile([128, RW], F16, tag="pxyr", name="pxyr")
                q1r = prod.tile([128, RW], F16, tag="q1r", name="q1r")
                q2r = prod.tile([128, RW], F16, tag="q2r", name="q2r")
                nc.vector.tensor_tensor(pxxr[:, 0:514], ixm[:, 0:514],
                                        ixm[:, 0:514], ALU.mult)
                nc.vector.tensor_tensor(pyyr[:, 0:514], iym[:, 0:514],
                                        iym[:, 0:514], ALU.mult)
                nc.vector.tensor_tensor(pxyr[:, 0:514], ixm[:, 0:514],
                                        iym[:, 0:514], ALU.mult)
                nc.vector.tensor_tensor(q1r[:, 0:514], pxxr[:, 0:514],
                                        pyyr[:, 0:514], ALU.add)
                nc.vector.tensor_tensor(q2r[:, 0:514], pxxr[:, 0:514],
                                        pyyr[:, 0:514], ALU.subtract)
                aixr = outp.tile([128, RW], F16, tag="aixr", name="aixr")
                aiyr = outp.tile([128, RW], F16, tag="aiyr", name="aiyr")
                nc.vector.tensor_scalar(
                    aixr[:, 0:514].bitcast(mybir.dt.uint16),
                    ixm[:, 0:514].bitcast(mybir.dt.uint16),
                    0x7FFF, None, ALU.bitwise_and)
                nc.vector.tensor_scalar(
                    aiyr[:, 0:514].bitcast(mybir.dt.uint16),
                    iym[:, 0:514].bitcast(mybir.dt.uint16),
                    0x7FFF, None, ALU.bitwise_and)
                edger = outp.tile([128, RW], F16, tag="edger", name="edger")
                _tt(edge_eng, edger[:, 0:514], aixr[:, 0:514],
                    aiyr[:, 0:514], ALU.add)
                eigr = outp.tile([128, RW], F16, tag="eigr", name="eigr")
                box_tail(q1r, q2r, pxyr, R_BOX_CHUNKS, eigr)
                nc.sync.dma_start(out=redge_o[:, :], in_=edger[:, 1:513])
                nc.sync.dma_start(out=reig_o[:, :], in_=eigr[:, 0:512])

            # ----------------- 4 full tiles x 2 stripes -------------------
            for t in range(NT):
                for s in range(NSTRIPE):
                    col0 = SIMG * s
                    xt = [xpool.tile([128, SW], F16, tag=f"x{c}", name=f"x{c}")
                          for c in range(2)]
                    for c in range(2):
                        nc.sync.dma_start(
                            out=xt[c][:], in_=xs[c, t, :, col0:col0 + SW])
                    if mode == "dmaonly":
                        nc.sync.dma_start(out=edge_o[t, s, :, :],
                                          in_=zed[:, 0:SIMG])
                        nc.sync.dma_start(out=eig_o[t, s, :, :],
                                          in_=zed[:, 0:SIMG])
                        continue
                    gray_t = gpool.tile([128, SW], F16, tag="gray", name="gray")
                    _tt(gray_eng, gray_t[:, 0:2052], xt[0][:, 0:2052],
                        xt[1][:, 0:2052], ALU.add)
                    gxy = gpool.tile([128, 2, SW], F16, tag="gxy", name="gxy")
                    grads(gray_t, wsb, GRAD_CHUNKS, gxy)
                    ix = gxy[:, 0, :]
                    iy = gxy[:, 1, :]
                    # zero the out-of-image product column (stripe0: image
                    # col -1 at idx 0; stripe1: col 4096 at idx 2049)
                    bc = 0 if s == 0 else 2049
                    nc.vector.memset(gxy[:, 0, bc:bc + 1], 0.0)
                    nc.vector.memset(gxy[:, 1, bc:bc + 1], 0.0)
                    pxx = prod.tile([128, SW], F16, tag="pxx", name="pxx")
                    pyy = prod.tile([128, SW], F16, tag="pyy", name="pyy")
                    pxy = prod.tile([128, SW], F16, tag="pxy", name="pxy")
                    q1 = prod.tile([128, SW], F16, tag="q1", name="q1")
                    q2 = prod.tile([128, SW], F16, tag="q2", name="q2")
                    nc.vector.tensor_tensor(pxx[:, 0:2050], ix[:, 0:2050],
                                            ix[:, 0:2050], ALU.mult)
                    nc.vector.tensor_tensor(pyy[:, 0:2050], iy[:, 0:2050],
                                            iy[:, 0:2050], ALU.mult)
                    nc.gpsimd.tensor_tensor(pxy[:, 0:2050], ix[:, 0:2050],
                                            iy[:, 0:2050], ALU.mult)
                    nc.vector.tensor_tensor(q1[:, 0:2050], pxx[:, 0:2050],
                                            pyy[:, 0:2050], ALU.add)
                    nc.vector.tensor_tensor(q2[:, 0:2050], pxx[:, 0:2050],
                                            pyy[:, 0:2050], ALU.subtract)
                    aix = outp.tile([128, SW], F16, tag="aix", name="aix")
                    aiy = outp.tile([128, SW], F16, tag="aiy", name="aiy")
                    nc.vector.tensor_scalar(
                        aix[:, 0:2050].bitcast(mybir.dt.uint16),
                        ix[:, 0:2050].bitcast(mybir.dt.uint16),
                        0x7FFF, None, ALU.bitwise_and)
                    nc.vector.tensor_scalar(
                        aiy[:, 0:2050].bitcast(mybir.dt.uint16),
                        iy[:, 0:2050].bitcast(mybir.dt.uint16),
                        0x7FFF, None, ALU.bitwise_and)
                    edge_sb = outp.tile([128, SW], F16, tag="edge", name="edge")
                    _tt(edge_eng, edge_sb[:, 0:2050], aix[:, 0:2050],
                        aiy[:, 0:2050], ALU.add)
                    nc.sync.dma_start(out=edge_o[t, s, :, :],
                                      in_=edge_sb[:, 1:2049])
                    s2q1 = prod.tile([128, SW], F16, tag="s2q1", name="s2q1")
                    nc.gpsimd.tensor_tensor(s2q1[:, 0:2048], q1[:, 0:2048],
                                            q1[:, 2:2050], ALU.add)
                    eig_sb = outp.tile([128, SW], F16, tag="eig", name="eig")
                    box_tail(q1, q2, pxy, BOX_CHUNKS, eig_sb, s2q1)
                    nc.sync.dma_start(out=eig_o[t, s, :, :],
                                      in_=eig_sb[:, 0:2048])
    return nc


_NC_CACHE = {}


def _get_nc(mode="full"):
    if mode not in _NC_CACHE:
        _NC_CACHE[mode] = build_nc(mode=mode)
    return _NC_CACHE[mode]


def _in_maps(x):
    """Per-core input dicts from the full [1,3,H,W] f32 image.
    Host packs channels to two planes: x01 = (w0*x0 + w1*x1)/w2 and x2,
    so device gray' = x01 + x2 = gray/w2 (w2 folded into grad weights)."""
    gxp = np.zeros((2, H, PW), dtype=np.float16)
    gxp[0, :, 2:2 + W] = (x[0, 0] * np.float32(S0 * S1)
                          + x[0, 1] * np.float32(S1)).astype(np.float16)
    gxp[1, :, 2:2 + W] = x[0, 2]
    wts = _weights()
    maps = []
    for k in range(N_CORES):
        xs = np.empty((2, NT, 128, PW), dtype=np.float16)
        for t in range(NT):
            j = 4 * k + t
            xs[:, t] = gxp[:, 124 * j:124 * j + 128, :]
        xrr = np.ascontiguousarray(gxp[:, 3968:4096, 512 * k:512 * k + RW])
        rm = np.ones((128, RW), dtype=np.float16)
        if k == 0:
            rm[:, 0] = 0.0
        if k == N_CORES - 1:
            rm[:, 513] = 0.0
        m = {"xs": xs, "xr": xrr, "rmask": rm}
        m.update(wts)
        maps.append(m)
    return maps


def kernel(x, edge_filter):
    x = np.asarray(x, dtype=np.float32)
    nc = _get_nc()
    maps = _in_maps(x)
    res = run_bass_kernel_spmd(nc, maps, list(range(N_CORES)))
    edge = np.empty((1, H, W), dtype=np.float32)
    eig = np.empty((1, H, W), dtype=np.float32)
    for k in range(N_CORES):
        r = res.results[k]
        for t in range(NT):
            j = 4 * k + t
            plo = 0 if j == 0 else 2
            rows = slice(124 * j + plo, 124 * j + 126)
            for s in range(NSTRIPE):
                cols = slice(SIMG * s, SIMG * (s + 1))
                edge[0, rows, cols] = r["edge"][t, s, plo:126].astype(np.float32)
                eig[0, rows, cols] = r["eig"][t, s, plo:126].astype(np.float32)
        cols = slice(512 * k, 512 * k + 512)
        edge[0, 3970:4096, cols] = r["redge"][2:128].astype(np.float32)
        eig[0, 3970:4096, cols] = r["reig"][2:128].astype(np.float32)
    return (edge, eig)
